# revision 9
# baseline (speedup 1.0000x reference)
"""Causal single-head attention (B=4, S=4096, D=1024, H=128) on 8 NeuronCores.

Sharding: core c = (batch b = c//2, half h = c%2). Each core:
  - computes K^T [h, 4096] and V [4096, 4] for its full batch row (replicated
    across the 2 cores of a batch),
  - handles 2048 query rows: 16 parity-interleaved 128-row subtiles
    (global subtile g = 8*r + 2*s + h for slot r in 0..3, s in 0..3),
  - slots have uniform causal k-tile limits [8, 16, 24, 32] so all 8 cores run
    the identical compiled program; causality is enforced with per-core mask
    DATA (qpos vs kiota is_ge compare) on the last 8 k-iters of each slot.

Pipeline per core (all matmuls bf16 with fp32 PSUM accumulate):
  x (bf16, host-cast) --DMA-xbar-transpose--> x^T tiles [d, s]
  K^T/V^T/Q^T = W.T @ x^T   (+bias via ACT Identity on PSUM->SBUF copy)
  V natural via PE transpose of V^T
  S^T[k, q] = (K^T)^T-brick.T @ Q^T  (K=h=128, N=512)
  P^T = exp(S^T / sqrt(H))  (ACT, bf16 out; no max subtraction - scores ~ +-2.5)
  causal mask: P^T *= (qpos >= kpos)  on last-8 k-iters
  O^T[h, q] += V-brick.T @ P^T ; d[1, q] += ones.T @ P^T   (PSUM accumulate)
  O = transpose(O^T) * 1/(d*sqrt(H))  -> DMA out
"""

import numpy as np
import ml_dtypes
from contextlib import ExitStack

import concourse.bass as bass
import concourse.tile as tile
from concourse import bacc, mybir
from concourse.bass_utils import run_bass_kernel_spmd

B, S, D, H = 4, 4096, 1024, 128
P = 128
BF16 = mybir.dt.bfloat16
F32 = mybir.dt.float32
NPBF16 = ml_dtypes.bfloat16

QLOC = 2048          # query rows per core
NSLOT = 4            # slots per core
SLOT_W = 512         # q columns per slot
LIMITS = [8, 16, 24, 32]   # k-tile limit per slot (same for every core)
NKT = S // P         # 32 k tiles
DCH = D // P         # 8 contraction chunks
SCALE = 1.0 / float(np.sqrt(H))     # pre-exp scale
POSTSCALE = float(np.sqrt(H))       # folded into denominator


def qglob_for_core(h):
    """Global query row indices (length QLOC) handled by core-half h, in local order."""
    idx = []
    for r in range(NSLOT):
        for s in range(4):
            g = 8 * r + 2 * s + h
            idx.append(np.arange(g * P, (g + 1) * P))
    return np.concatenate(idx)


def build_nc():
    nc = bacc.Bacc(None, target_bir_lowering=False, debug=False, num_devices=8)

    xt = nc.dram_tensor("xt", [D, S], BF16, kind="ExternalInput").ap()
    xqt = nc.dram_tensor("xqt", [D, QLOC], BF16, kind="ExternalInput").ap()
    w_ap = {}
    for nm in ("wq", "wk", "wv"):
        w_ap[nm] = nc.dram_tensor(nm, [D, H], BF16, kind="ExternalInput").ap()
    b_ap = {}
    for nm in ("bq", "bk", "bv"):
        b_ap[nm] = nc.dram_tensor(nm, [H, 1], F32, kind="ExternalInput").ap()
    qpos = nc.dram_tensor("qpos", [1, QLOC], F32, kind="ExternalInput").ap()
    kio = nc.dram_tensor("kio", [P, NKT], F32, kind="ExternalInput").ap()
    identb = nc.dram_tensor("identb", [P, P], BF16, kind="ExternalInput").ap()
    identf = nc.dram_tensor("identf", [P, P], F32, kind="ExternalInput").ap()
    onesb = nc.dram_tensor("onesb", [P, 1], BF16, kind="ExternalInput").ap()
    out = nc.dram_tensor("out", [QLOC, H], F32, kind="ExternalOutput").ap()

    Ident = mybir.ActivationFunctionType.Identity
    Copy = mybir.ActivationFunctionType.Copy
    Exp = mybir.ActivationFunctionType.Exp

    with tile.TileContext(nc) as tc, ExitStack() as ctx:
        consts = ctx.enter_context(tc.tile_pool(name="consts", bufs=1))
        persist = ctx.enter_context(tc.tile_pool(name="persist", bufs=1))

        # ---- constants into SBUF
        w_sb = {}
        for nm in ("wq", "wk", "wv"):
            t = consts.tile([P, DCH, H], BF16, tag=f"w_{nm}")
            nc.sync.dma_start(out=t[:], in_=w_ap[nm].rearrange("(c p) h -> p c h", p=P))
            w_sb[nm] = t
        b_sb = {}
        for nm in ("bq", "bk", "bv"):
            t = consts.tile([P, 1], F32, tag=f"b_{nm}")
            nc.sync.dma_start(out=t[:], in_=b_ap[nm])
            b_sb[nm] = t
        qpos_b = consts.tile([P, QLOC], F32, tag="qpos_b")
        nc.gpsimd.dma_start(
            out=qpos_b[:],
            in_=bass.AP(tensor=qpos.tensor, offset=qpos.offset, ap=[[0, P], [1, QLOC]]),
        )
        kio_sb = consts.tile([P, NKT], F32, tag="kio")
        nc.sync.dma_start(out=kio_sb[:], in_=kio)
        identb_sb = consts.tile([P, P], BF16, tag="identb")
        nc.sync.dma_start(out=identb_sb[:], in_=identb)
        identf_sb = consts.tile([P, P], F32, tag="identf")
        nc.sync.dma_start(out=identf_sb[:], in_=identf)
        ones_sb = consts.tile([P, 1], BF16, tag="ones")
        nc.sync.dma_start(out=ones_sb[:], in_=onesb)

        # ---- persistent activations
        kT = persist.tile([P, S], BF16, tag="kT")          # K^T [h, s]
        vN = persist.tile([P, NKT, H], BF16, tag="vN")     # V natural [k_l, kt, h]
        qT = persist.tile([P, QLOC], BF16, tag="qT")       # Q^T [h, q_local]
        xt_sb = persist.tile([P, DCH, S], BF16, tag="xt_sb")    # x^T resident
        xqt_sb = persist.tile([P, DCH, QLOC], BF16, tag="xqt_sb")

        # ---- phase A: K^T, V from x^T; Q^T from xq^T
        with tc.tile_pool(name="stg", bufs=3) as stg, \
             tc.tile_pool(name="psA", bufs=3, space="PSUM") as psA, \
             tc.tile_pool(name="psV", bufs=2, space="PSUM") as psV:

            for j in range(DCH):
                for hlf in range(2):
                    nc.sync.dma_start(
                        out=xt_sb[:, j, hlf * (S // 2):(hlf + 1) * (S // 2)],
                        in_=xt[j * P:(j + 1) * P, hlf * (S // 2):(hlf + 1) * (S // 2)],
                    )
                nc.sync.dma_start(
                    out=xqt_sb[:, j, :],
                    in_=xqt[j * P:(j + 1) * P, :],
                )

            def project(src_sb, sr, wname):
                ps = psA.tile([P, SLOT_W], F32, tag="proj")
                for j in range(DCH):
                    nc.tensor.matmul(
                        ps[:], lhsT=w_sb[wname][:, j, :],
                        rhs=src_sb[:, j, sr * SLOT_W:(sr + 1) * SLOT_W],
                        start=(j == 0), stop=(j == DCH - 1),
                    )
                return ps

            for sr in range(S // SLOT_W):
                # K^T stripe
                ps = project(xt_sb, sr, "wk")
                nc.scalar.activation(
                    kT[:, sr * SLOT_W:(sr + 1) * SLOT_W], ps[:], Ident,
                    bias=b_sb["bk"][:], scale=1.0,
                )
                # V^T stripe -> V natural bricks
                ps2 = project(xt_sb, sr, "wv")
                vTs = stg.tile([P, SLOT_W], BF16, tag="vT")
                nc.scalar.activation(vTs[:], ps2[:], Ident, bias=b_sb["bv"][:], scale=1.0)
                pst = psV.tile([P, SLOT_W], BF16, tag="vtr")
                for t_ in range(4):
                    nc.tensor.matmul(
                        pst[:, t_ * P:(t_ + 1) * P], lhsT=vTs[:, t_ * P:(t_ + 1) * P],
                        rhs=identb_sb[:], is_transpose=True, skip_group_check=True,
                    )
                nc.scalar.activation(vN[:, sr * 4:(sr + 1) * 4, :], pst[:], Copy)
                # interleave Q^T stripes so attention slot 0 can start early
                if sr % 2 == 0:
                    qr = sr // 2
                    psq = project(xqt_sb, qr, "wq")
                    nc.scalar.activation(
                        qT[:, qr * SLOT_W:(qr + 1) * SLOT_W], psq[:], Ident,
                        bias=b_sb["bq"][:], scale=1.0,
                    )

        # ---- phase B: attention
        with tc.tile_pool(name="psS", bufs=2, space="PSUM") as psS, \
             tc.tile_pool(name="psO", bufs=2, space="PSUM") as psO, \
             tc.tile_pool(name="psD", bufs=1, space="PSUM") as psD, \
             tc.tile_pool(name="psE", bufs=1, space="PSUM") as psE, \
             tc.tile_pool(name="pp", bufs=3) as pp, \
             tc.tile_pool(name="epi", bufs=2) as epi:

            for r in range(NSLOT):
                L = LIMITS[r]
                qsl = slice(r * SLOT_W, (r + 1) * SLOT_W)
                oT = psO.tile([P, SLOT_W], F32, tag="oT")
                dacc = psD.tile([1, SLOT_W], F32, tag="dacc")

                def score(kt):
                    """S^T = K-brick.T @ Q^T, exp -> P^T (bf16), causal mask."""
                    sT = psS.tile([P, SLOT_W], F32, tag="sT")
                    nc.tensor.matmul(
                        sT[:], lhsT=kT[:, kt * P:(kt + 1) * P], rhs=qT[:, qsl],
                        start=True, stop=True,
                    )
                    pT = pp.tile([P, SLOT_W], BF16, tag="pT")
                    nc.scalar.activation(pT[:], sT[:], Exp, scale=SCALE)
                    if kt >= L - 8:
                        msk = pp.tile([P, SLOT_W], BF16, tag="msk")
                        nc.vector.tensor_scalar(
                            msk[:], qpos_b[:, qsl], kio_sb[:, kt:kt + 1], None,
                            op0=mybir.AluOpType.is_ge,
                        )
                        nc.vector.tensor_mul(pT[:], pT[:], msk[:])
                    return pT

                def accum(kt, pT):
                    nc.tensor.matmul(
                        oT[:], lhsT=vN[:, kt, :], rhs=pT[:],
                        start=(kt == 0), stop=(kt == L - 1),
                    )
                    nc.tensor.matmul(
                        dacc[:], lhsT=ones_sb[:], rhs=pT[:],
                        start=(kt == 0), stop=(kt == L - 1),
                    )

                # software pipeline: PV/d trail score/exp by one iteration so the
                # tensor engine never stalls waiting on the scalar engine's exp
                pT_prev = score(0)
                for kt in range(1, L):
                    pT = score(kt)
                    accum(kt - 1, pT_prev)
                    pT_prev = pT
                accum(L - 1, pT_prev)
                # epilogue: O = transpose(O^T) / (d * sqrt(H))
                oTs = epi.tile([P, SLOT_W], F32, tag="oTs")
                nc.scalar.activation(oTs[:], oT[:], Copy)
                ds_ = epi.tile([1, SLOT_W], F32, tag="ds")
                nc.scalar.activation(ds_[:], dacc[:], Copy, scale=POSTSCALE)
                dT = psE.tile([P, 4], F32, tag="dT")
                for s_ in range(4):
                    nc.tensor.matmul(
                        dT[:, s_:s_ + 1], lhsT=ds_[:, s_ * P:(s_ + 1) * P],
                        rhs=identf_sb[0:1, 0:1], is_transpose=True,
                        skip_group_check=True,
                    )
                rec = epi.tile([P, 4], F32, tag="rec")
                nc.vector.reciprocal(rec[:], dT[:])
                obr = psE.tile([P, SLOT_W], F32, tag="obr")
                for s_ in range(4):
                    nc.tensor.matmul(
                        obr[:, s_ * P:(s_ + 1) * P], lhsT=oTs[:, s_ * P:(s_ + 1) * P],
                        rhs=identf_sb[:], is_transpose=True, skip_group_check=True,
                    )
                ofin = epi.tile([P, SLOT_W], F32, tag="ofin")
                for s_ in range(4):
                    nc.vector.tensor_scalar_mul(
                        ofin[:, s_ * P:(s_ + 1) * P], obr[:, s_ * P:(s_ + 1) * P],
                        rec[:, s_:s_ + 1],
                    )
                nc.sync.dma_start(
                    out=out[r * SLOT_W:(r + 1) * SLOT_W, :].rearrange(
                        "(s p) h -> p s h", p=P
                    ),
                    in_=ofin[:].rearrange("p (s h) -> p s h", s=4),
                )

    nc.compile()
    return nc


_NC_CACHE = None


def _get_nc():
    global _NC_CACHE
    if _NC_CACHE is None:
        _NC_CACHE = build_nc()
    return _NC_CACHE


def make_in_maps(inputs):
    x = np.asarray(inputs["x"], np.float32)
    Wq = np.asarray(inputs["Wq"], np.float32)
    Wk = np.asarray(inputs["Wk"], np.float32)
    Wv = np.asarray(inputs["Wv"], np.float32)
    bq = np.asarray(inputs["bq"], np.float32)
    bk = np.asarray(inputs["bk"], np.float32)
    bv = np.asarray(inputs["bv"], np.float32)

    xb = x.astype(NPBF16)
    kio = (np.arange(NKT)[None, :] * P + np.arange(P)[:, None]).astype(np.float32)
    common = dict(
        wq=Wq.astype(NPBF16), wk=Wk.astype(NPBF16), wv=Wv.astype(NPBF16),
        bq=bq.reshape(H, 1), bk=bk.reshape(H, 1), bv=bv.reshape(H, 1),
        kio=kio,
        identb=np.eye(P, dtype=NPBF16),
        identf=np.eye(P, dtype=np.float32),
        onesb=np.ones((P, 1), dtype=NPBF16),
    )
    in_maps = []
    xbT = np.ascontiguousarray(xb.transpose(0, 2, 1))  # [B, D, S]
    for c in range(8):
        b, hh = c // 2, c % 2
        qg = qglob_for_core(hh)
        m = dict(common)
        m["xt"] = xbT[b]
        m["xqt"] = np.ascontiguousarray(xbT[b][:, qg])
        m["qpos"] = qg.astype(np.float32).reshape(1, QLOC)
        in_maps.append(m)
    return in_maps


def assemble_out(results):
    out = np.zeros((1, B, S, H), np.float32)
    for c in range(8):
        b, hh = c // 2, c % 2
        qg = qglob_for_core(hh)
        out[0, b, qg, :] = results[c]["out"]
    return out


def kernel(**inputs) -> np.ndarray:
    nc = _get_nc()
    in_maps = make_in_maps(inputs)
    res = run_bass_kernel_spmd(nc, in_maps, list(range(8)))
    return assemble_out(res.results)
